# revision 1
# baseline (speedup 1.0000x reference)
"""GCN encoder (2-layer, PyG GCNConv w/ self-loops + symmetric norm) on 8 trn2 cores.

Math per layer: out = dis * ((A+I)(dis*x)) @ W + b, with dis = deg^-1/2, which
factorizes the per-edge norm dis[s]*dis[d] into a source row pre-scale and a
destination row post-scale (no per-edge scalar work).

Device pipeline per core (destinations row-sharded, 49 blocks of 128 rows):
  prep:   x' = bf16(dis * x), replicated on every core (A/B half tensors)
  layer:  dma_gather 256B source rows per edge (block-grouped, padded chunks)
          -> PE segment-sum: one-hot S (built by a 2x-mode is_equal) x msgs
          -> two source-half phases (partial f32 accumulator bridges them)
          -> epilogue: dis[d] scale, ^T, @W, bias(+relu), write
  between layers: two AllGathers (B half early, A half late) so the layer-2
          B-phase gathers overlap the A-half AllGather.
Destinations are degree-balance-permuted into blocks (host un-permutes the
output), minimizing gather-chunk padding.
"""

import sys

sys.path.insert(0, "/opt/trn_rl_repo")

import numpy as np
import ml_dtypes

BF16 = ml_dtypes.bfloat16

D = 128
P = 8


def _sizes(n):
    rpc = -(-n // (P * 128)) * 128  # rows per core, multiple of 128
    npad = rpc * P
    b = rpc // 128  # dest blocks per core
    nt = npad // 128
    gs = 1
    for d_ in range(1, 9):
        if b % d_ == 0:
            gs = d_
    g = b // gs
    ba = ((g + 1) // 2) * gs if g >= 2 else b  # A-half blocks, group-aligned
    ra, rb = ba * 128, (b - ba) * 128
    return rpc, npad, b, nt, gs, ba, ra, rb


def plan(edge_index, n):
    """Host-side integer preprocessing.

    Destinations are permuted into degree-balanced 128-row blocks (round-robin
    over blocks by descending degree) so every (block, src-half) has a near-
    equal edge count -> minimal chunk padding. Layer 1 gathers from the
    original-order x'; layer 2 gathers from the permuted-order activations,
    so each layer gets its own index/drel tables.
    """
    rpc, npad, b, nt, gs, ba, ra, rb = _sizes(n)
    nblocks = P * b
    src = edge_index[0].astype(np.int64)
    dst = edge_index[1].astype(np.int64)
    loops = np.arange(n, dtype=np.int64)
    allsrc = np.concatenate([src, loops])
    alldst = np.concatenate([dst, loops])

    deg = np.bincount(alldst, minlength=n).astype(np.float32)
    deg_pad = np.ones(npad, dtype=np.float32)
    deg_pad[:n] = deg
    deg_t = np.ascontiguousarray(deg_pad.reshape(nt, 128).T)  # [128, nt] orig order

    # degree-balanced destination permutation: node -> padded row
    by_deg = np.argsort(-deg, kind="stable")
    bid = np.arange(n, dtype=np.int64) % nblocks
    slot = np.arange(n, dtype=np.int64) // nblocks
    perm_row = np.empty(n, dtype=np.int64)
    perm_row[by_deg] = (bid // b) * rpc + (bid % b) * 128 + slot
    degrow = np.ones(npad, dtype=np.float32)
    degrow[perm_row] = deg
    degrow_t = np.ascontiguousarray(degrow.reshape(nt, 128).T)  # [128, nt] permuted

    dst_row = perm_row[alldst]
    core = dst_row // rpc
    dloc = dst_row - core * rpc
    blk = dloc >> 7
    drel = (dloc & 127).astype(np.float32)

    # per-layer source row mappings into the A/B halves
    s_core1 = allsrc // rpc
    s_w1 = allsrc - s_core1 * rpc
    hi1 = (s_w1 >= ra).astype(np.int64)
    sidx1 = np.where(hi1 == 0, s_core1 * ra + s_w1, s_core1 * rb + (s_w1 - ra))
    src_row2 = perm_row[allsrc]
    s_core2 = src_row2 // rpc
    s_w2 = src_row2 - s_core2 * rpc
    hi2 = (s_w2 >= ra).astype(np.int64)
    sidx2 = np.where(hi2 == 0, s_core2 * ra + s_w2, s_core2 * rb + (s_w2 - ra))
    assert sidx1.max() < 32768 and sidx2.max() < 32768

    g = b // gs
    nkeys = P * b * 2

    def slotize(sidx, hi):
        key = (core * b + blk) * 2 + hi
        counts = np.bincount(key, minlength=nkeys)
        cc = counts.reshape(P, b, 2)
        k2lo = max(1, int(-(-cc[:, :, 0].max() // 128)))
        k2hi = max(1, int(-(-cc[:, :, 1].max() // 128)))
        spl, sph = k2lo * 128, k2hi * 128
        order = np.argsort(key, kind="stable")
        key_s = key[order]
        run_start = np.zeros(nkeys, dtype=np.int64)
        np.cumsum(counts[:-1], out=run_start[1:])
        rank = np.arange(key_s.size, dtype=np.int64) - run_start[key_s]
        slot_base = np.zeros(nkeys + 1, dtype=np.int64)
        per_key = np.where(np.arange(nkeys) % 2 == 0, spl, sph)
        np.cumsum(per_key, out=slot_base[1:])
        pos = slot_base[key_s] + rank
        tot = int(slot_base[-1])
        idx_flat = np.zeros(tot, dtype=np.int16)
        drel_flat = np.full(tot, -1.0, dtype=np.float32)
        idx_flat[pos] = sidx[order].astype(np.int16)
        drel_flat[pos] = drel[order]
        per_blk = idx_flat.reshape(P, b, spl + sph)
        drel_blk = drel_flat.reshape(P, b, spl + sph)
        k2t = k2lo + k2hi
        L_lo, L_hi = gs * spl, gs * sph

        def make_idx(core_slots, L):
            l16 = L // 16
            seq = core_slots.reshape(g, L)  # [g, L] block-major
            tile = seq.reshape(g, l16, 16).transpose(0, 2, 1)
            tile = np.tile(tile, (1, 8, 1))
            return np.ascontiguousarray(
                tile.transpose(1, 0, 2).reshape(128, g * l16)
            )

        idx_lo = [make_idx(per_blk[c, :, :spl], L_lo) for c in range(P)]
        idx_hi = [make_idx(per_blk[c, :, spl:], L_hi) for c in range(P)]
        drs = []
        for c in range(P):
            dr = drel_blk[c].reshape(b, k2t, 128).transpose(2, 0, 1)
            drs.append(np.ascontiguousarray(dr.reshape(128, b * k2t)).astype(BF16))
        return dict(
            k2lo=k2lo, k2hi=k2hi, k2t=k2t, l16lo=L_lo // 16, l16hi=L_hi // 16,
            idx_lo=idx_lo, idx_hi=idx_hi, drel=drs,
        )

    lay1 = slotize(sidx1, hi1)
    lay2 = slotize(sidx2, hi2)
    k2tmax = max(lay1["k2t"], lay2["k2t"])

    per_core = []
    for c in range(P):
        deg_own = np.ascontiguousarray(degrow_t[:, c * b : (c + 1) * b])
        per_core.append(
            {
                "idx_lo1": lay1["idx_lo"][c], "idx_hi1": lay1["idx_hi"][c],
                "idx_lo2": lay2["idx_lo"][c], "idx_hi2": lay2["idx_hi"][c],
                "drel1": lay1["drel"][c], "drel2": lay2["drel"][c],
                "deg_own": deg_own,
            }
        )

    # iota_rep[p, j*k2tmax + c] = j  (chunk-minor layout for 2x-mode is_equal)
    iota_rep = np.repeat(np.arange(128, dtype=np.float32), k2tmax)
    iota_rep = np.tile(iota_rep, (128, 1)).astype(BF16)
    ident = np.eye(128, dtype=np.float32)
    lk = tuple(
        (la["k2lo"], la["k2hi"], la["l16lo"], la["l16hi"]) for la in (lay1, lay2)
    )
    return {
        "sizes": (rpc, npad, b, nt, gs, ba, ra, rb, g, k2tmax) + lk,
        "deg_t": deg_t,
        "per_core": per_core,
        "perm_row": perm_row,
        "iota_rep": iota_rep,
        "ident_bf": ident.astype(BF16),
        "ident_f32": ident,
    }


def build_program(pl):
    import concourse.mybir as mybir
    from concourse.bacc import Bacc
    from concourse.tile import TileContext

    (rpc, npad, b, nt, gs, ba, ra, rb, g, k2tmax, lk1, lk2) = pl["sizes"]
    na, nb = P * ra, P * rb
    ga = ba // gs  # groups in the A half
    f32 = mybir.dt.float32
    bf16 = mybir.dt.bfloat16
    i16 = mybir.dt.int16
    AF = mybir.ActivationFunctionType
    OP = mybir.AluOpType

    nc = Bacc(num_devices=P)

    x_in = nc.declare_dram_parameter("x", [npad, D], f32, isOutput=False)
    degt_in = nc.declare_dram_parameter("deg_t", [128, nt], f32, isOutput=False)
    dego_in = nc.declare_dram_parameter("deg_own", [128, b], f32, isOutput=False)
    w1_in = nc.declare_dram_parameter("W1", [D, D], f32, isOutput=False)
    b1_in = nc.declare_dram_parameter("b1", [D, 1], f32, isOutput=False)
    w2_in = nc.declare_dram_parameter("W2", [D, D], f32, isOutput=False)
    b2_in = nc.declare_dram_parameter("b2", [D, 1], f32, isOutput=False)
    b2t_in = nc.declare_dram_parameter("b2_tile", [D, D], f32, isOutput=False)
    iota_in = nc.declare_dram_parameter("iota_rep", [128, 128 * k2tmax], bf16, isOutput=False)
    identb_in = nc.declare_dram_parameter("ident_bf", [128, 128], bf16, isOutput=False)
    identf_in = nc.declare_dram_parameter("ident_f32", [128, 128], f32, isOutput=False)
    lay_in = []
    for li, (k2lo, k2hi, l16lo, l16hi) in ((1, lk1), (2, lk2)):
        lay_in.append(
            (
                nc.declare_dram_parameter(
                    f"idx_lo{li}", [128, g * l16lo], i16, isOutput=False
                ),
                nc.declare_dram_parameter(
                    f"idx_hi{li}", [128, g * l16hi], i16, isOutput=False
                ),
                nc.declare_dram_parameter(
                    f"drel{li}", [128, b * (k2lo + k2hi)], bf16, isOutput=False
                ),
            )
        )
    out = nc.declare_dram_parameter("out", [rpc, D], f32, isOutput=True)

    split = rb > 0
    x1a = nc.dram_tensor("x1a", [na, D], bf16)
    x2own_a = nc.dram_tensor("x2own_a", [ra, D], bf16)
    x2lo = nc.dram_tensor("x2lo", [na, D], bf16, addr_space="Shared")
    if split:
        x1b = nc.dram_tensor("x1b", [nb, D], bf16)
        x2own_b = nc.dram_tensor("x2own_b", [rb, D], bf16)
        x2hi = nc.dram_tensor("x2hi", [nb, D], bf16, addr_space="Shared")
    else:
        x1b, x2own_b, x2hi = x1a, None, x2lo

    with TileContext(nc) as tc:
        with (
            tc.tile_pool(name="const", bufs=1) as const,
            tc.tile_pool(name="prep", bufs=4) as prep,
            tc.tile_pool(name="msgs", bufs=3) as msgs,
            tc.tile_pool(name="spool", bufs=4) as spool,
            tc.tile_pool(name="yout", bufs=3) as yout,
            tc.tile_pool(name="epi", bufs=8) as epi,
            tc.tile_pool(name="pa", bufs=4, space="PSUM") as pa,
            tc.tile_pool(name="pt", bufs=2, space="PSUM") as pt,
            tc.tile_pool(name="pz", bufs=2, space="PSUM") as pz,
        ):
            # ---- constants -------------------------------------------------
            def load_const(param, shape, dtype, tag):
                t = const.tile(shape, dtype, tag=tag)
                nc.sync.dma_start(t[:], param[:])
                return t

            degt_sb = load_const(degt_in, [128, nt], f32, "degt")
            dego_sb = load_const(dego_in, [128, b], f32, "dego")
            w1_sb = load_const(w1_in, [D, D], f32, "w1")
            w2_sb = load_const(w2_in, [D, D], f32, "w2")
            b1_sb = load_const(b1_in, [D, 1], f32, "b1")
            b2_sb = load_const(b2_in, [D, 1], f32, "b2")
            b2t_sb = load_const(b2t_in, [D, D], f32, "b2t")
            iota_sb = load_const(iota_in, [128, 128 * k2tmax], bf16, "iota")
            identb_sb = load_const(identb_in, [128, 128], bf16, "identb")
            identf_sb = load_const(identf_in, [128, 128], f32, "identf")
            lay_sb = []
            for li, (k2lo, k2hi, l16lo, l16hi) in ((0, lk1), (1, lk2)):
                ilo, ihi, drl = lay_in[li]
                lay_sb.append(
                    (
                        load_const(ilo, [128, g * l16lo], i16, f"idxlo{li}"),
                        load_const(ihi, [128, g * l16hi], i16, f"idxhi{li}"),
                        load_const(drl, [128, b * (k2lo + k2hi)], bf16, f"drel{li}"),
                    )
                )

            rec_t = const.tile([128, nt], f32, tag="rec_t")
            nc.vector.reciprocal(rec_t[:], degt_sb[:])
            dis_t = const.tile([128, nt], f32, tag="dis_t")
            nc.scalar.activation(dis_t[:], rec_t[:], AF.Sqrt)
            rec_o = const.tile([128, b], f32, tag="rec_o")
            nc.vector.reciprocal(rec_o[:], dego_sb[:])
            dis_o = const.tile([128, b], f32, tag="dis_o")
            nc.scalar.activation(dis_o[:], rec_o[:], AF.Sqrt)

            w1b = const.tile([D, D], bf16, tag="w1b")
            nc.vector.tensor_copy(w1b[:], w1_sb[:])
            w2b = const.tile([D, D], bf16, tag="w2b")
            nc.vector.tensor_copy(w2b[:], w2_sb[:])

            # ---- prep: x1{a,b} = bf16(x * dis), batched --------------------
            def prep_run(tile0, ntiles, dstt, drow0):
                # process `ntiles` consecutive 128-row tiles starting at
                # global tile `tile0`, writing to dstt rows starting drow0
                off = 0
                while off < ntiles:
                    ch = min(14, ntiles - off)
                    t0 = tile0 + off
                    xt = prep.tile([128, 14, D], f32, tag="xt")
                    nc.sync.dma_start(
                        xt[:, 0:ch, :],
                        x_in[t0 * 128 : (t0 + ch) * 128, :].rearrange(
                            "(a p) d -> p a d", p=128
                        ),
                    )
                    xb = prep.tile([128, 14, D], bf16, tag="xb")
                    for i in range(ch):
                        nc.scalar.activation(
                            xb[:, i, :],
                            xt[:, i, :],
                            AF.Copy,
                            scale=dis_t[:, t0 + i : t0 + i + 1],
                        )
                    r0 = drow0 + off * 128
                    nc.sync.dma_start(
                        dstt[r0 : r0 + ch * 128, :].rearrange("(a p) d -> p a d", p=128),
                        xb[:, 0:ch, :],
                    )
                    off += ch

            for sc in range(P):
                prep_run(sc * b, ba, x1a, sc * ra)
            if split:
                for sc in range(P):
                    prep_run(sc * b + ba, b - ba, x1b, sc * rb)

            # ---- one GCN layer, two source-half phases --------------------
            # phase A accumulates dis[d]*sum(msgs of one half) into partial;
            # phase B adds the other half, then runs the block epilogue.
            partial = const.tile([128, b * 128], f32, tag="partial")

            def halves(lay, h):
                k2lo, k2hi, l16lo, l16hi = (lk1, lk2)[lay]
                ilo, ihi, drl = lay_sb[lay]
                k2t = k2lo + k2hi
                if h == 0:
                    return ilo, 16 * l16lo, l16lo, k2lo, 0, k2t, drl
                return ihi, 16 * l16hi, l16hi, k2hi, k2lo, k2t, drl

            def gather_half(gg, src, lay, h):
                idx_sb, L, l16, k2h, _, _, _ = halves(lay, h)
                msg = msgs.tile([128, gs * k2h, D], bf16, tag="msg")
                nc.gpsimd.dma_gather(
                    msg[:, :, :],
                    src,
                    idx_sb[:, gg * l16 : (gg + 1) * l16],
                    L,
                    L,
                    D,
                    single_packet=False,
                )
                return msg

            def block_agg(j, bb, msg, lay, h):
                _, _, _, k2h, koff, k2t, drel_sb = halves(lay, h)
                S = spool.tile([128, 128, k2tmax], bf16, tag="S")
                nc.vector.tensor_tensor(
                    S[:, :, 0:k2h],
                    iota_sb[:, :]
                    .rearrange("p (j c) -> p j c", j=128)[:, :, 0:k2h],
                    drel_sb[:, bb * k2t + koff : bb * k2t + koff + k2h]
                    .rearrange("p (a c) -> p a c", a=1)
                    .broadcast_to([128, 128, k2h]),
                    OP.is_equal,
                )
                agg = pa.tile([128, D], f32, tag="agg")
                for k in range(k2h):
                    nc.tensor.matmul(
                        agg[:],
                        S[:, :, k],
                        msg[:, j * k2h + k, :],
                        start=(k == 0),
                        stop=(k == k2h - 1),
                    )
                return agg

            def do_phase_a(gg, src, lay, h):
                msg = gather_half(gg, src, lay, h)
                for j in range(gs):
                    bb = gg * gs + j
                    agg = block_agg(j, bb, msg, lay, h)
                    nc.scalar.activation(
                        partial[:, bb * 128 : (bb + 1) * 128],
                        agg[:],
                        AF.Copy,
                        scale=dis_o[:, bb : bb + 1],
                    )

            def do_phase_b(gg, src, lay, h, first):
                wb = w1b if first else w2b
                msg = gather_half(gg, src, lay, h)
                ystage = yout.tile([128, gs, D], bf16 if first else f32, tag="yst")
                for j in range(gs):
                    bb = gg * gs + j
                    agg = block_agg(j, bb, msg, lay, h)
                    # aggs = dis[d]*agg + partial  (bf16)
                    aggs = epi.tile([128, D], bf16, tag="aggs")
                    nc.vector.scalar_tensor_tensor(
                        aggs[:],
                        agg[:],
                        dis_o[:, bb : bb + 1],
                        partial[:, bb * 128 : (bb + 1) * 128],
                        OP.mult,
                        OP.add,
                    )
                    aggT_p = pt.tile([128, D], bf16, tag="aggT_p")
                    nc.tensor.transpose(aggT_p[:], aggs[:], identb_sb[:])
                    aggT = epi.tile([128, D], bf16, tag="aggT")
                    nc.scalar.activation(aggT[:], aggT_p[:], AF.Copy)
                    if first:
                        z_p = pz.tile([128, D], f32, tag="z_p")
                        nc.tensor.matmul(
                            z_p[:], wb[:], aggT[:], start=True, stop=True
                        )
                        zs = epi.tile([128, D], bf16, tag="zs")
                        nc.scalar.activation(
                            zs[:], z_p[:], AF.Relu, bias=b1_sb[:, 0:1]
                        )
                        y_p = pz.tile([128, D], bf16, tag="z_p")
                        nc.tensor.transpose(y_p[:], zs[:], identb_sb[:])
                        nc.vector.tensor_scalar(
                            ystage[:, j, :], y_p[:], dis_o[:, bb : bb + 1], None, OP.mult
                        )
                    else:
                        # direct [dest, dhid] = aggT.T @ W, then + b2 tile
                        z_p = pz.tile([128, D], f32, tag="z_p")
                        nc.tensor.matmul(
                            z_p[:], aggT[:], wb[:], start=True, stop=True
                        )
                        nc.vector.scalar_tensor_tensor(
                            ystage[:, j, :], z_p[:], 1.0, b2t_sb[:], OP.mult, OP.add
                        )
                b0 = gg * gs
                if first:
                    if b0 >= ba:
                        r0 = (b0 - ba) * 128
                        nc.sync.dma_start(
                            x2own_b[r0 : r0 + gs * 128, :].rearrange(
                                "(a p) d -> p a d", p=128
                            ),
                            ystage[:, :, :],
                        )
                    else:
                        nc.sync.dma_start(
                            x2own_a[b0 * 128 : (b0 + gs) * 128, :].rearrange(
                                "(a p) d -> p a d", p=128
                            ),
                            ystage[:, :, :],
                        )
                else:
                    nc.sync.dma_start(
                        out[b0 * 128 : (b0 + gs) * 128, :].rearrange(
                            "(a p) d -> p a d", p=128
                        ),
                        ystage[:, :, :],
                    )

            # layer 1: B-half dest groups first so AG_B can start early
            for gg in range(ga, g):
                do_phase_a(gg, x1a[:, :], 0, 0)
            for gg in range(ga, g):
                do_phase_b(gg, x1b[:, :], 0, 1, True)
            if split:
                nc.gpsimd.collective_compute(
                    "AllGather",
                    mybir.AluOpType.bypass,
                    replica_groups=[list(range(P))],
                    ins=[x2own_b[:]],
                    outs=[x2hi[:]],
                )
            for gg in range(ga):
                do_phase_a(gg, x1a[:, :], 0, 0)
            for gg in range(ga):
                do_phase_b(gg, x1b[:, :], 0, 1, True)
            nc.gpsimd.collective_compute(
                "AllGather",
                mybir.AluOpType.bypass,
                replica_groups=[list(range(P))],
                ins=[x2own_a[:]],
                outs=[x2lo[:]],
            )
            # layer 2: hi phase first (needs only AG_B), lo merge phase after AG_A
            for gg in range(g):
                do_phase_a(gg, x2hi[:, :], 1, 1)
            for gg in range(g):
                do_phase_b(gg, x2lo[:, :], 1, 0, False)

    nc.finalize()
    return nc


def make_in_maps(pl, x, w1, b1, w2, b2):
    n = x.shape[0]
    npad = pl["sizes"][1]
    x_pad = np.zeros((npad, D), dtype=np.float32)
    x_pad[:n] = x
    shared = {
        "x": x_pad,
        "deg_t": pl["deg_t"],
        "W1": np.ascontiguousarray(w1.astype(np.float32)),
        "b1": np.ascontiguousarray(b1.astype(np.float32).reshape(D, 1)),
        "W2": np.ascontiguousarray(w2.astype(np.float32)),
        "b2": np.ascontiguousarray(b2.astype(np.float32).reshape(D, 1)),
        "b2_tile": np.ascontiguousarray(
            np.tile(b2.astype(np.float32).reshape(1, D), (D, 1))
        ),
        "iota_rep": pl["iota_rep"],
        "ident_bf": pl["ident_bf"],
        "ident_f32": pl["ident_f32"],
    }
    in_maps = []
    for c in range(P):
        m = dict(shared)
        pc = pl["per_core"]
        for kk in ("deg_own", "idx_lo1", "idx_hi1", "idx_lo2", "idx_hi2",
                   "drel1", "drel2"):
            m[kk] = pc[c][kk]
        in_maps.append(m)
    return in_maps


_CACHE = {}


def kernel(x, edge_index, W1, b1, W2, b2):
    from concourse.bass_utils import run_bass_kernel_spmd

    x = np.asarray(x)
    edge_index = np.asarray(edge_index)
    n = x.shape[0]
    pl = plan(edge_index, n)
    key = pl["sizes"]
    if key not in _CACHE:
        _CACHE[key] = build_program(pl)
    nc = _CACHE[key]
    in_maps = make_in_maps(
        pl, x, np.asarray(W1), np.asarray(b1), np.asarray(W2), np.asarray(b2)
    )
    last_err = None
    for backoff in (15.0, 45.0, 0.0):
        try:
            r = run_bass_kernel_spmd(nc, in_maps, list(range(P)))
            break
        except Exception as ex:  # transient NRT/axon failures wedge briefly
            last_err = ex
            if backoff:
                import time

                time.sleep(backoff)
    else:
        raise last_err
    outs = np.concatenate([r.results[c]["out"] for c in range(P)], axis=0)
    return np.ascontiguousarray(outs[pl["perm_row"][:n]]).astype(np.float32)



# revision 13
# speedup vs baseline: 1.3614x; 1.3614x over previous
"""GCN encoder (2-layer, PyG GCNConv w/ self-loops + symmetric norm) on 8 trn2 cores.

Math per layer: out = dis * ((A+I)(dis*x)) @ W + b, with dis = deg^-1/2, which
factorizes the per-edge norm dis[s]*dis[d] into a source row pre-scale and a
destination row post-scale.

Host side: x' = bf16(dis*x) is computed on host (input transform, like the
index tables), so the device does no prep pass. Destinations are permuted
into degree-balanced 128-row blocks (host un-permutes the output).

Device pipeline per core (destinations row-sharded, 49 blocks of 128 rows):
  layer 1: dma_gather 256B source rows per edge from x' (block-grouped,
           lo/hi split at row 32768 for int16 idx range)
           -> PE segment-sum transposed: stat=msg chunk, mov=one-hot S
              (built by 2x-mode is_equal) accumulating aggT[d, j] in PSUM
           -> epilogue (feature-major): z = W1^T aggT; transpose;
              x2 = dis*relu(dis*z + b1) via one DVE STT + one Act relu-scale
           -> x2own chunk write (partition-major layout, big DMA descs)
  exchange: K=4 chunked AllGathers of x2own (bf16), pipelined against L1
           production and L2 consumption (COLLECTIVE_CORES runs serially)
  layer 2: per AG chunk k: gathers from x2chunk_k; PSUM per (block, region),
           merged into an f32 SBUF partial; final epilogue per block:
           out = dis*(W2^T agg)^T + b2, written partition-major (host
           un-permutes).
"""

import sys

sys.path.insert(0, "/opt/trn_rl_repo")

import numpy as np
import ml_dtypes

BF16 = ml_dtypes.bfloat16

D = 128
P = 8
NCHUNK = 4
LO_SPLIT = 32768  # layer-1 source row split for int16 gather indices


def _sizes(n):
    rpc = -(-n // (P * 128)) * 128  # rows per core, multiple of 128
    npad = rpc * P
    b = rpc // 128  # dest blocks per core
    gs = 1
    for d_ in range(1, 9):
        if b % d_ == 0:
            gs = d_
    g = b // gs
    # AG chunk boundaries in blocks (first chunks take the remainder)
    base = b // NCHUNK
    rem = b - base * NCHUNK
    cb = []
    b0 = 0
    for k in range(NCHUNK):
        bk = base + (1 if k < rem else 0)
        cb.append((b0, b0 + bk))
        b0 += bk
    return rpc, npad, b, gs, g, cb


def plan(edge_index, n):
    """Host-side preprocessing: degree-balanced dest permutation, slot tables.

    Layer 1 gathers from x' (original node order), slots split lo/hi at row
    LO_SPLIT. Layer 2 gathers from the AG chunk tensors x2chunk_k; a source
    node's chunk is determined by its permuted block position.
    """
    rpc, npad, b, gs, g, cb = _sizes(n)
    nblocks = P * b
    src = edge_index[0].astype(np.int64)
    dst = edge_index[1].astype(np.int64)
    loops = np.arange(n, dtype=np.int64)
    allsrc = np.concatenate([src, loops])
    alldst = np.concatenate([dst, loops])

    deg = np.bincount(alldst, minlength=n).astype(np.float32)
    dis = 1.0 / np.sqrt(np.maximum(deg, 1.0))

    # degree-balanced destination permutation: node -> (core, block, slot)
    by_deg = np.argsort(-deg, kind="stable")
    bid = np.arange(n, dtype=np.int64) % nblocks
    slot = np.arange(n, dtype=np.int64) // nblocks
    perm_core = np.empty(n, dtype=np.int64)
    perm_blk = np.empty(n, dtype=np.int64)
    perm_slot = np.empty(n, dtype=np.int64)
    perm_core[by_deg] = bid // b
    perm_blk[by_deg] = bid % b
    perm_slot[by_deg] = slot

    # dis in permuted partition-major layout: dis_pm[core][p, b]
    dis_pm = np.zeros((P, 128, b), dtype=np.float32)
    dis_pm[perm_core, perm_slot, perm_blk] = dis

    dst_core = perm_core[alldst]
    dst_blk = perm_blk[alldst]
    drel = perm_slot[alldst].astype(np.float32)

    # layer-1 source rows: original order; lo/hi split
    hi1 = (allsrc >= LO_SPLIT).astype(np.int64)
    sidx1 = np.where(hi1 == 0, allsrc, allsrc - LO_SPLIT)

    # layer-2 source rows: x2chunk_k flat index
    # node (c, blk, p) in chunk k (cb[k][0] <= blk < cb[k][1]) at
    # flat row c*(128*bk) + p*bk + (blk - b0k)
    s_core = perm_core[allsrc]
    s_blk = perm_blk[allsrc]
    s_slot = perm_slot[allsrc]
    chunk_of_blk = np.zeros(b, dtype=np.int64)
    for k, (b0, b1) in enumerate(cb):
        chunk_of_blk[b0:b1] = k
    reg2 = chunk_of_blk[s_blk]
    bks = np.array([b1 - b0 for (b0, b1) in cb], dtype=np.int64)
    b0s = np.array([b0 for (b0, b1) in cb], dtype=np.int64)
    sidx2 = s_core * (128 * bks[reg2]) + s_slot * bks[reg2] + (s_blk - b0s[reg2])
    assert sidx1.max() < 32768 and sidx2.max() < 32768

    def slotize(sidx, cls, ncls, nedges=None):
        """Pack edges into per-(core, destblock, class) slot chunks.

        Returns per-core idx tables (one per class, block-grouped) and the
        drel table covering all classes' chunks consecutively. nedges
        restricts to the first nedges entries (drops self-loops for L2).
        """
        dc, db, dr_, sx, cl = dst_core, dst_blk, drel, sidx, cls
        if nedges is not None:
            dc, db, dr_ = dc[:nedges], db[:nedges], dr_[:nedges]
            sx, cl = sx[:nedges], cl[:nedges]
        key = (dc * b + db) * ncls + cl
        nkeys = P * b * ncls
        counts = np.bincount(key, minlength=nkeys)
        cc = counts.reshape(P, b, ncls)
        k2 = [max(1, int(-(-cc[:, :, j].max() // 128))) for j in range(ncls)]
        sp = [kk * 128 for kk in k2]
        order = np.argsort(key, kind="stable")
        key_s = key[order]
        run_start = np.zeros(nkeys, dtype=np.int64)
        np.cumsum(counts[:-1], out=run_start[1:])
        rank = np.arange(key_s.size, dtype=np.int64) - run_start[key_s]
        slot_base = np.zeros(nkeys + 1, dtype=np.int64)
        per_key = np.array([sp[j] for j in range(ncls)] * (P * b), dtype=np.int64)
        np.cumsum(per_key, out=slot_base[1:])
        pos = slot_base[key_s] + rank
        tot = int(slot_base[-1])
        idx_flat = np.zeros(tot, dtype=np.int16)
        drel_flat = np.full(tot, -1.0, dtype=np.float32)
        idx_flat[pos] = sx[order].astype(np.int16)
        drel_flat[pos] = dr_[order]
        spt = sum(sp)
        per_blk = idx_flat.reshape(P, b, spt)
        drel_blk = drel_flat.reshape(P, b, spt)
        k2t = sum(k2)

        def make_idx(core_slots, L):
            # [g, L] block-major slot lists -> [128, g*L/16] idx tile layout
            l16 = L // 16
            seq = core_slots.reshape(g, L)
            tile = seq.reshape(g, l16, 16).transpose(0, 2, 1)
            tile = np.tile(tile, (1, 8, 1))
            return np.ascontiguousarray(tile.transpose(1, 0, 2).reshape(128, g * l16))

        idx_cls = []  # per class: list per core of idx tables
        off = 0
        for j in range(ncls):
            L = gs * sp[j]
            idx_cls.append(
                [
                    make_idx(per_blk[c, :, off : off + sp[j]], L)
                    for c in range(P)
                ]
            )
            off += sp[j]
        drs = []
        for c in range(P):
            dr = drel_blk[c].reshape(b, k2t, 128).transpose(2, 0, 1)
            drs.append(np.ascontiguousarray(dr.reshape(128, b * k2t)).astype(BF16))
        return dict(k2=k2, k2t=k2t, idx=idx_cls, drel=drs)

    lay1 = slotize(sidx1, hi1, 2)
    # L2 drops self-loops (handled on-device via local transpose-accumulate)
    lay2 = slotize(sidx2, reg2, NCHUNK, nedges=src.size)
    k2tmax = max(lay1["k2t"], lay2["k2t"])

    per_core = []
    for c in range(P):
        per_core.append(
            {
                "idx1_lo": lay1["idx"][0][c],
                "idx1_hi": lay1["idx"][1][c],
                "drel1": lay1["drel"][c],
                "drel2": lay2["drel"][c],
                "dis_pm": np.ascontiguousarray(dis_pm[c]),
                **{f"idx2_{k}": lay2["idx"][k][c] for k in range(NCHUNK)},
            }
        )

    # iota_rep[p, j*k2tmax + c] = j  (chunk-minor for 2x-mode is_equal)
    iota_rep = np.repeat(np.arange(128, dtype=np.float32), k2tmax)
    iota_rep = np.tile(iota_rep, (128, 1)).astype(BF16)
    ident = np.eye(128, dtype=np.float32)
    return {
        "sizes": (rpc, npad, b, gs, g, tuple(cb), k2tmax,
                  tuple(lay1["k2"]), tuple(lay2["k2"])),
        "per_core": per_core,
        "perm": (perm_core, perm_blk, perm_slot),
        "dis": dis,
        "iota_rep": iota_rep,
        "ident_bf": ident.astype(BF16),
    }


def build_program(pl):
    import concourse.mybir as mybir
    from concourse.bacc import Bacc
    from concourse.tile import TileContext

    (rpc, npad, b, gs, g, cb, k2tmax, k2l1, k2l2) = pl["sizes"]
    f32 = mybir.dt.float32
    bf16 = mybir.dt.bfloat16
    i16 = mybir.dt.int16
    AF = mybir.ActivationFunctionType
    OP = mybir.AluOpType
    k2t1 = sum(k2l1)
    k2t2 = sum(k2l2)

    nc = Bacc(num_devices=P)

    xp_in = nc.declare_dram_parameter("xp", [npad, D], bf16, isOutput=False)
    dis_in = nc.declare_dram_parameter("dis_pm", [128, b], f32, isOutput=False)
    w1_in = nc.declare_dram_parameter("W1", [D, D], f32, isOutput=False)
    w2_in = nc.declare_dram_parameter("W2", [D, D], f32, isOutput=False)
    b1t_in = nc.declare_dram_parameter("b1_tile", [D, D], f32, isOutput=False)
    b2t_in = nc.declare_dram_parameter("b2_tile", [D, D], f32, isOutput=False)
    iota_in = nc.declare_dram_parameter(
        "iota_rep", [128, 128 * k2tmax], bf16, isOutput=False
    )
    identb_in = nc.declare_dram_parameter("ident_bf", [128, 128], bf16, isOutput=False)
    idx1lo_in = nc.declare_dram_parameter(
        "idx1_lo", [128, g * gs * k2l1[0] * 8], i16, isOutput=False
    )
    idx1hi_in = nc.declare_dram_parameter(
        "idx1_hi", [128, g * gs * k2l1[1] * 8], i16, isOutput=False
    )
    idx2_in = [
        nc.declare_dram_parameter(
            f"idx2_{k}", [128, g * gs * k2l2[k] * 8], i16, isOutput=False
        )
        for k in range(NCHUNK)
    ]
    drel1_in = nc.declare_dram_parameter("drel1", [128, b * k2t1], bf16, isOutput=False)
    drel2_in = nc.declare_dram_parameter("drel2", [128, b * k2t2], bf16, isOutput=False)
    out = nc.declare_dram_parameter("out", [128, b, D], f32, isOutput=True)

    # AG chunk tensors: in = [128, bk, 128] partition-major; out stacks cores
    x2own = []
    x2chunk = []
    for k, (b0, b1) in enumerate(cb):
        bk = b1 - b0
        x2own.append(nc.dram_tensor(f"x2own_{k}", [128, bk, D], bf16))
        x2chunk.append(
            nc.dram_tensor(f"x2chunk_{k}", [P, 128, bk, D], bf16, addr_space="Shared")
        )

    with TileContext(nc) as tc:
        with (
            tc.tile_pool(name="const", bufs=1) as const,
            tc.tile_pool(name="msgs", bufs=3) as msgs,
            tc.tile_pool(name="spool", bufs=3) as spool,
            tc.tile_pool(name="yout", bufs=3) as yout,
            tc.tile_pool(name="epi", bufs=8) as epi,
            tc.tile_pool(name="pa", bufs=4, space="PSUM") as pa,
            tc.tile_pool(name="pt", bufs=2, space="PSUM") as pt,
            tc.tile_pool(name="pz", bufs=2, space="PSUM") as pz,
        ):
            # ---- constants -------------------------------------------------
            def load_const(param, shape, dtype, tag):
                t = const.tile(shape, dtype, tag=tag)
                nc.sync.dma_start(t[:], param[:])
                return t

            dis_sb = load_const(dis_in, [128, b], f32, "dis")
            w1_sb = load_const(w1_in, [D, D], f32, "w1")
            w2_sb = load_const(w2_in, [D, D], f32, "w2")
            b1t_sb = load_const(b1t_in, [D, D], f32, "b1t")
            b2t_sb = load_const(b2t_in, [D, D], f32, "b2t")
            iota_sb = load_const(iota_in, [128, 128 * k2tmax], bf16, "iota")
            identb_sb = load_const(identb_in, [128, 128], bf16, "identb")
            idx1lo_sb = load_const(
                idx1lo_in, [128, g * gs * k2l1[0] * 8], i16, "idx1lo"
            )
            idx1hi_sb = load_const(
                idx1hi_in, [128, g * gs * k2l1[1] * 8], i16, "idx1hi"
            )
            idx2_sb = [
                load_const(idx2_in[k], [128, g * gs * k2l2[k] * 8], i16, f"idx2_{k}")
                for k in range(NCHUNK)
            ]
            drel1_sb = load_const(drel1_in, [128, b * k2t1], bf16, "drel1")
            drel2_sb = load_const(drel2_in, [128, b * k2t2], bf16, "drel2")

            w1b = const.tile([D, D], bf16, tag="w1b")
            nc.vector.tensor_copy(w1b[:], w1_sb[:])
            w2b = const.tile([D, D], bf16, tag="w2b")
            nc.vector.tensor_copy(w2b[:], w2_sb[:])

            # f32 partial for layer-2 aggregation (aggT layout [d, j] per block)
            partial = const.tile([128, b * 128], f32, tag="partial")

            def gather(idx_sb, src_ap, k2h, gg):
                L = gs * k2h * 128
                l16 = L // 16
                msg = msgs.tile([128, gs * k2h, D], bf16, tag="msg")
                nc.gpsimd.dma_gather(
                    msg[:, :, :],
                    src_ap,
                    idx_sb[:, gg * l16 : (gg + 1) * l16],
                    L,
                    L,
                    D,
                    single_packet=False,
                )
                return msg

            def build_S(drel_sb, k2t, bb, koff=0, nch=None):
                # one-hot S slice for chunks [koff, koff+nch) of block bb
                if nch is None:
                    nch = k2t
                S = spool.tile([128, 128, k2tmax], bf16, tag="S")
                nc.vector.tensor_tensor(
                    S[:, :, 0:nch],
                    iota_sb[:, :].rearrange("p (j c) -> p j c", j=128)[:, :, 0:nch],
                    drel_sb[:, bb * k2t + koff : bb * k2t + koff + nch]
                    .rearrange("p (a c) -> p a c", a=1)
                    .broadcast_to([128, 128, nch]),
                    OP.is_equal,
                )
                return S

            # ---- layer 1 ---------------------------------------------------
            # per group: gather lo+hi, per block: S, 19 matmuls -> aggT PSUM,
            # epilogue -> x2stage [j, hid] bf16, write per chunk
            k2lo, k2hi = k2l1

            def l1_group(gg):
                mlo = gather(idx1lo_sb, xp_in[0:LO_SPLIT, :], k2lo, gg)
                mhi = gather(idx1hi_sb, xp_in[LO_SPLIT:npad, :], k2hi, gg)
                ystage = yout.tile([128, gs, D], bf16, tag="yst")
                for j in range(gs):
                    bb = gg * gs + j
                    S = build_S(drel1_sb, k2t1, bb)
                    aggT = pa.tile([128, D], f32, tag="aggT")
                    for c in range(k2lo):
                        nc.tensor.matmul(
                            aggT[:],
                            mlo[:, j * k2lo + c, :],
                            S[:, :, c],
                            start=(c == 0),
                            stop=False,
                        )
                    for c in range(k2hi):
                        nc.tensor.matmul(
                            aggT[:],
                            mhi[:, j * k2hi + c, :],
                            S[:, :, k2lo + c],
                            start=False,
                            stop=(c == k2hi - 1),
                        )
                    # z = W1^T aggT  [hid, j]
                    aggb = epi.tile([128, D], bf16, tag="aggb")
                    nc.scalar.activation(aggb[:], aggT[:], AF.Copy)
                    z_p = pz.tile([128, D], f32, tag="z_p")
                    nc.tensor.matmul(z_p[:], w1b[:], aggb[:], start=True, stop=True)
                    zs = epi.tile([128, D], bf16, tag="zs")
                    nc.scalar.activation(zs[:], z_p[:], AF.Copy)
                    zT_p = pt.tile([128, D], bf16, tag="zT_p")
                    nc.tensor.transpose(zT_p[:], zs[:], identb_sb[:])
                    # tmp = dis_j * zT + b1_tile ; x2 = relu(dis_j * tmp)
                    tmp = epi.tile([128, D], f32, tag="tmp")
                    nc.vector.scalar_tensor_tensor(
                        tmp[:],
                        zT_p[:],
                        dis_sb[:, bb : bb + 1],
                        b1t_sb[:],
                        OP.mult,
                        OP.add,
                    )
                    nc.scalar.activation(
                        ystage[:, j, :],
                        tmp[:],
                        AF.Relu,
                        scale=dis_sb[:, bb : bb + 1],
                    )
                # write ystage blocks into their chunks (partition-major)
                b0g = gg * gs
                j0 = 0
                while j0 < gs:
                    blk = b0g + j0
                    k = next(i for i, (c0, c1) in enumerate(cb) if c0 <= blk < c1)
                    c0, c1 = cb[k]
                    take = min(gs - j0, c1 - blk)
                    nc.sync.dma_start(
                        x2own[k][:, blk - c0 : blk - c0 + take, :],
                        ystage[:, j0 : j0 + take, :],
                    )
                    j0 += take

            # ---- layer 2 region pass --------------------------------------
            def load_own_group(gg):
                # own block rows (self-loop sources), [128, gs, D] bf16
                xgrp = yout.tile([128, gs, D], bf16, tag="xgrp")
                b0g = gg * gs
                j0 = 0
                while j0 < gs:
                    blk = b0g + j0
                    k = next(i for i, (c0, c1) in enumerate(cb) if c0 <= blk < c1)
                    c0, c1 = cb[k]
                    take = min(gs - j0, c1 - blk)
                    nc.sync.dma_start(
                        xgrp[:, j0 : j0 + take, :],
                        x2own[k][:, blk - c0 : blk - c0 + take, :],
                    )
                    j0 += take
                return xgrp

            def l2_region(k, gg, first):
                k2r = k2l2[k]
                koff = sum(k2l2[:k])
                src = x2chunk[k][:, :, :, :].rearrange("c p b d -> (c p b) d")
                m = gather(idx2_sb[k], src, k2r, gg)
                xgrp = load_own_group(gg) if first else None
                for j in range(gs):
                    bb = gg * gs + j
                    S = build_S(drel2_sb, k2t2, bb, koff=koff, nch=k2r)
                    aggT = pa.tile([128, D], f32, tag="aggT")
                    if first:
                        # self-loop contribution: aggT[d, j'] += xgrp[j', d]
                        # (stat=xgrp, mov=identity == transpose, f32 accum)
                        nc.tensor.matmul(
                            aggT[:],
                            xgrp[:, j, :],
                            identb_sb[:],
                            start=True,
                            stop=False,
                        )
                    for c in range(k2r):
                        nc.tensor.matmul(
                            aggT[:],
                            m[:, j * k2r + c, :],
                            S[:, :, koff + c],
                            start=(not first) and (c == 0),
                            stop=(c == k2r - 1),
                        )
                    if first:
                        nc.scalar.activation(
                            partial[:, bb * 128 : (bb + 1) * 128], aggT[:], AF.Copy
                        )
                    else:
                        nc.vector.scalar_tensor_tensor(
                            partial[:, bb * 128 : (bb + 1) * 128],
                            aggT[:],
                            1.0,
                            partial[:, bb * 128 : (bb + 1) * 128],
                            OP.mult,
                            OP.add,
                        )

            def l2_final(gg):
                ostage = yout.tile([128, gs, D], f32, tag="ost")
                for j in range(gs):
                    bb = gg * gs + j
                    aggb = epi.tile([128, D], bf16, tag="aggb")
                    nc.vector.tensor_copy(
                        aggb[:], partial[:, bb * 128 : (bb + 1) * 128]
                    )
                    z_p = pz.tile([128, D], f32, tag="z_p")
                    nc.tensor.matmul(z_p[:], w2b[:], aggb[:], start=True, stop=True)
                    zs = epi.tile([128, D], bf16, tag="zs")
                    nc.scalar.activation(zs[:], z_p[:], AF.Copy)
                    zT_p = pt.tile([128, D], bf16, tag="zT_p")
                    nc.tensor.transpose(zT_p[:], zs[:], identb_sb[:])
                    nc.vector.scalar_tensor_tensor(
                        ostage[:, j, :],
                        zT_p[:],
                        dis_sb[:, bb : bb + 1],
                        b2t_sb[:],
                        OP.mult,
                        OP.add,
                    )
                nc.sync.dma_start(
                    out[:, gg * gs : (gg + 1) * gs, :], ostage[:, :, :]
                )

            # ---- schedule: L1 groups, AGs as chunks complete, L2 regions --
            done_chunk = [False] * NCHUNK
            blocks_done = 0
            for gg in range(g):
                l1_group(gg)
                blocks_done += gs
                for k, (c0, c1) in enumerate(cb):
                    if not done_chunk[k] and blocks_done >= c1:
                        nc.gpsimd.collective_compute(
                            "AllGather",
                            mybir.AluOpType.bypass,
                            replica_groups=[list(range(P))],
                            ins=[x2own[k][:, :, :]],
                            outs=[x2chunk[k][:, :, :, :]],
                        )
                        done_chunk[k] = True
            for k in range(NCHUNK):
                for gg in range(g):
                    l2_region(k, gg, first=(k == 0))
            for gg in range(g):
                l2_final(gg)

    nc.finalize()
    return nc


def make_in_maps(pl, x, w1, b1, w2, b2):
    n = x.shape[0]
    (rpc, npad, b, gs, g, cb, k2tmax, k2l1, k2l2) = pl["sizes"]
    dis = pl["dis"]
    xp = np.zeros((npad, D), dtype=BF16)
    xp[:n] = (x.astype(np.float32) * dis[:, None]).astype(BF16)
    shared = {
        "xp": xp,
        "W1": np.ascontiguousarray(w1.astype(np.float32)),
        "W2": np.ascontiguousarray(w2.astype(np.float32)),
        "b1_tile": np.ascontiguousarray(
            np.tile(b1.astype(np.float32).reshape(1, D), (D, 1))
        ),
        "b2_tile": np.ascontiguousarray(
            np.tile(b2.astype(np.float32).reshape(1, D), (D, 1))
        ),
        "iota_rep": pl["iota_rep"],
        "ident_bf": pl["ident_bf"],
    }
    in_maps = []
    for c in range(P):
        m = dict(shared)
        pc = pl["per_core"][c]
        m["dis_pm"] = pc["dis_pm"]
        m["idx1_lo"] = pc["idx1_lo"]
        m["idx1_hi"] = pc["idx1_hi"]
        m["drel1"] = pc["drel1"]
        m["drel2"] = pc["drel2"]
        for k in range(NCHUNK):
            m[f"idx2_{k}"] = pc[f"idx2_{k}"]
        in_maps.append(m)
    return in_maps


_CACHE = {}


def kernel(x, edge_index, W1, b1, W2, b2):
    from concourse.bass_utils import run_bass_kernel_spmd

    x = np.asarray(x)
    edge_index = np.asarray(edge_index)
    n = x.shape[0]
    pl = plan(edge_index, n)
    key = pl["sizes"]
    if key not in _CACHE:
        _CACHE[key] = build_program(pl)
    nc = _CACHE[key]
    in_maps = make_in_maps(
        pl, x, np.asarray(W1), np.asarray(b1), np.asarray(W2), np.asarray(b2)
    )
    last_err = None
    for backoff in (15.0, 45.0, 0.0):
        try:
            r = run_bass_kernel_spmd(nc, in_maps, list(range(P)))
            break
        except Exception as ex:  # transient NRT/axon failures wedge briefly
            last_err = ex
            if backoff:
                import time

                time.sleep(backoff)
    else:
        raise last_err

    perm_core, perm_blk, perm_slot = pl["perm"]
    outs = np.stack([r.results[c]["out"] for c in range(P)], axis=0)
    # outs[c][p, b, d] -> node rows
    res = outs[perm_core, perm_slot, perm_blk]
    return np.ascontiguousarray(res).astype(np.float32)


# revision 55
# speedup vs baseline: 1.6652x; 1.2231x over previous
"""GCN encoder (2-layer, PyG GCNConv w/ self-loops + symmetric norm) on 8 trn2 cores.

Math per layer: out = dis * ((A+I)(dis*x)) @ W + b, with dis = deg^-1/2, which
factorizes the per-edge norm dis[s]*dis[d] into a source row pre-scale and a
destination row post-scale.

Host side: x' = bf16(dis*x) is computed on host (input transform, like the
index tables), so the device does no prep pass. Destinations are permuted
into degree-balanced 128-row blocks (host un-permutes the output).

Device pipeline per core (destinations row-sharded, 49 blocks of 128 rows):
  layer 1: dma_gather 256B source rows per edge from x' (block-grouped,
           lo/hi split at row 32768 for int16 idx range)
           -> PE segment-sum transposed: stat=msg chunk, mov=one-hot S
              (built by 2x-mode is_equal) accumulating aggT[d, j] in PSUM
           -> epilogue (feature-major): z = W1^T aggT; transpose;
              x2 = dis*relu(dis*z + b1) via one DVE STT + one Act relu-scale
           -> x2own chunk write (partition-major layout, big DMA descs)
  exchange: K=4 chunked AllGathers of x2own (bf16), pipelined against L1
           production and L2 consumption (COLLECTIVE_CORES runs serially)
  layer 2: per AG chunk k: gathers from x2chunk_k; PSUM per (block, region),
           merged into an f32 SBUF partial; final epilogue per block:
           out = dis*(W2^T agg)^T + b2, written partition-major (host
           un-permutes).
"""

import sys

sys.path.insert(0, "/opt/trn_rl_repo")

import numpy as np
import ml_dtypes

BF16 = ml_dtypes.bfloat16

D = 128
P = 8
NCHUNK = 4
LO_SPLIT = 32768  # layer-1 source row split for int16 gather indices
FP8X = False  # fp8 exchange corrupts data through the runtime AllGather; keep bf16
L2_DELAY_MS = 0.16


def _sizes(n):
    rpc = -(-n // (P * 128)) * 128  # rows per core, multiple of 128
    npad = rpc * P
    b = rpc // 128  # dest blocks per core
    gs = 1
    for d_ in range(1, 9):
        if b % d_ == 0:
            gs = d_
    g = b // gs
    # AG chunk boundaries in blocks: small first chunk (starts the serial
    # collective chain early) and small last chunk (short tail before the
    # final L2 region), big middle chunks.
    if b == 49:
        bks = [4, 12, 26, 7]
    else:
        base = b // NCHUNK
        rem = b - base * NCHUNK
        bks = [base + (1 if k < rem else 0) for k in range(NCHUNK)]
    cb = []
    b0 = 0
    for bk in bks:
        cb.append((b0, b0 + bk))
        b0 += bk
    return rpc, npad, b, gs, g, cb


def plan(edge_index, n):
    """Host-side preprocessing: degree-balanced dest permutation, slot tables.

    Layer 1 gathers from x' (original node order), slots split lo/hi at row
    LO_SPLIT. Layer 2 gathers from the AG chunk tensors x2chunk_k; a source
    node's chunk is determined by its permuted block position.
    """
    rpc, npad, b, gs, g, cb = _sizes(n)
    nblocks = P * b
    src = edge_index[0].astype(np.int64)
    dst = edge_index[1].astype(np.int64)
    loops = np.arange(n, dtype=np.int64)
    allsrc = np.concatenate([src, loops])
    alldst = np.concatenate([dst, loops])

    deg = np.bincount(alldst, minlength=n).astype(np.float32)
    dis = 1.0 / np.sqrt(np.maximum(deg, 1.0))
    outdeg = np.bincount(src, minlength=n).astype(np.float64)

    # Destination permutation: in-degree balanced level-by-level (level l =
    # the l-th in-degree rank round), with OUT-degree steered within each
    # level: the last AG chunk's blocks get the lowest-out-degree nodes (its
    # L2 gather region is the serial tail), the first chunk next (head), and
    # the big mid-chunk with the largest AG window absorbs the highest.
    by_deg = np.argsort(-deg, kind="stable")
    steer = [3, 0, 2, 1] if len(cb) == 4 else list(range(len(cb)))
    bid_order = np.array(
        [
            c * b + blk
            for k in steer
            for c in range(P)
            for blk in range(cb[k][0], cb[k][1])
        ],
        dtype=np.int64,
    )
    perm_core = np.empty(n, dtype=np.int64)
    perm_blk = np.empty(n, dtype=np.int64)
    perm_slot = np.empty(n, dtype=np.int64)
    pos = 0
    lvl = 0
    while pos < n:
        cnt = min(nblocks, n - pos)
        nodes = by_deg[pos : pos + cnt]
        if cnt == nblocks:
            nodes = nodes[np.argsort(outdeg[nodes], kind="stable")]
            ab = bid_order
        else:
            ab = np.arange(cnt, dtype=np.int64)
        perm_core[nodes] = ab[:cnt] // b
        perm_blk[nodes] = ab[:cnt] % b
        perm_slot[nodes] = lvl
        pos += cnt
        lvl += 1

    # dis in permuted partition-major layout: dis_pm[core][p, b]
    dis_pm = np.zeros((P, 128, b), dtype=np.float32)
    dis_pm[perm_core, perm_slot, perm_blk] = dis

    dst_core = perm_core[alldst]
    dst_blk = perm_blk[alldst]
    drel = perm_slot[alldst].astype(np.float32)

    # layer-1 source rows: original order; lo/hi split
    hi1 = (allsrc >= LO_SPLIT).astype(np.int64)
    sidx1 = np.where(hi1 == 0, allsrc, allsrc - LO_SPLIT)

    # layer-2 source rows: x2chunk_k flat index
    # node (c, blk, p) in chunk k (cb[k][0] <= blk < cb[k][1]) at
    # flat row c*(128*bk) + p*bk + (blk - b0k)
    s_core = perm_core[allsrc]
    s_blk = perm_blk[allsrc]
    s_slot = perm_slot[allsrc]
    chunk_of_blk = np.zeros(b, dtype=np.int64)
    for k, (b0, b1) in enumerate(cb):
        chunk_of_blk[b0:b1] = k
    reg2 = chunk_of_blk[s_blk]
    bks = np.array([b1 - b0 for (b0, b1) in cb], dtype=np.int64)
    b0s = np.array([b0 for (b0, b1) in cb], dtype=np.int64)
    sidx2 = s_core * (128 * bks[reg2]) + s_slot * bks[reg2] + (s_blk - b0s[reg2])
    assert sidx1.max() < 32768 and sidx2.max() < 32768

    def slotize(sidx, cls, ncls, nedges=None):
        """Pack edges into per-(core, destblock, class) slot chunks.

        Returns per-core idx tables (one per class, block-grouped) and the
        drel table covering all classes' chunks consecutively. nedges
        restricts to the first nedges entries (drops self-loops for L2).
        """
        dc, db, dr_, sx, cl = dst_core, dst_blk, drel, sidx, cls
        if nedges is not None:
            dc, db, dr_ = dc[:nedges], db[:nedges], dr_[:nedges]
            sx, cl = sx[:nedges], cl[:nedges]
        key = (dc * b + db) * ncls + cl
        nkeys = P * b * ncls
        counts = np.bincount(key, minlength=nkeys)
        cc = counts.reshape(P, b, ncls)
        k2 = [max(1, int(-(-cc[:, :, j].max() // 128))) for j in range(ncls)]
        sp = [kk * 128 for kk in k2]
        order = np.argsort(key, kind="stable")
        key_s = key[order]
        run_start = np.zeros(nkeys, dtype=np.int64)
        np.cumsum(counts[:-1], out=run_start[1:])
        rank = np.arange(key_s.size, dtype=np.int64) - run_start[key_s]
        slot_base = np.zeros(nkeys + 1, dtype=np.int64)
        per_key = np.array([sp[j] for j in range(ncls)] * (P * b), dtype=np.int64)
        np.cumsum(per_key, out=slot_base[1:])
        pos = slot_base[key_s] + rank
        tot = int(slot_base[-1])
        idx_flat = np.zeros(tot, dtype=np.int16)
        drel_flat = np.full(tot, -1.0, dtype=np.float32)
        idx_flat[pos] = sx[order].astype(np.int16)
        drel_flat[pos] = dr_[order]
        spt = sum(sp)
        per_blk = idx_flat.reshape(P, b, spt)
        drel_blk = drel_flat.reshape(P, b, spt)
        k2t = sum(k2)

        def make_idx(core_slots, L):
            # [g, L] block-major slot lists -> [128, g*L/16] idx tile layout
            l16 = L // 16
            seq = core_slots.reshape(g, L)
            tile = seq.reshape(g, l16, 16).transpose(0, 2, 1)
            tile = np.tile(tile, (1, 8, 1))
            return np.ascontiguousarray(tile.transpose(1, 0, 2).reshape(128, g * l16))

        idx_cls = []  # per class: list per core of idx tables
        off = 0
        for j in range(ncls):
            L = gs * sp[j]
            idx_cls.append(
                [
                    make_idx(per_blk[c, :, off : off + sp[j]], L)
                    for c in range(P)
                ]
            )
            off += sp[j]
        drs = []
        for c in range(P):
            dr = drel_blk[c].reshape(b, k2t, 128).transpose(2, 0, 1)
            drs.append(np.ascontiguousarray(dr.reshape(128, b * k2t)).astype(BF16))
        return dict(k2=k2, k2t=k2t, idx=idx_cls, drel=drs)

    lay1 = slotize(sidx1, hi1, 2)
    # L2 drops self-loops (handled on-device via local transpose-accumulate)
    lay2 = slotize(sidx2, reg2, NCHUNK, nedges=src.size)
    k2tmax = max(lay1["k2t"], lay2["k2t"])

    per_core = []
    for c in range(P):
        per_core.append(
            {
                "idx1_lo": lay1["idx"][0][c],
                "idx1_hi": lay1["idx"][1][c],
                "drel1": lay1["drel"][c],
                "drel2": lay2["drel"][c],
                "dis_pm": np.ascontiguousarray(dis_pm[c]),
                **{f"idx2_{k}": lay2["idx"][k][c] for k in range(NCHUNK)},
            }
        )

    # iota_rep[p, j*k2tmax + c] = j  (chunk-minor for 2x-mode is_equal)
    iota_rep = np.repeat(np.arange(128, dtype=np.float32), k2tmax)
    iota_rep = np.tile(iota_rep, (128, 1)).astype(BF16)
    ident = np.eye(128, dtype=np.float32)
    return {
        "sizes": (rpc, npad, b, gs, g, tuple(cb), k2tmax,
                  tuple(lay1["k2"]), tuple(lay2["k2"])),
        "per_core": per_core,
        "perm": (perm_core, perm_blk, perm_slot),
        "dis": dis,
        "iota_rep": iota_rep,
        "ident_bf": ident.astype(BF16),
    }


def build_program(pl):
    import concourse.mybir as mybir
    from concourse.bacc import Bacc
    from concourse.tile import TileContext

    (rpc, npad, b, gs, g, cb, k2tmax, k2l1, k2l2) = pl["sizes"]
    f32 = mybir.dt.float32
    bf16 = mybir.dt.bfloat16
    i16 = mybir.dt.int16
    AF = mybir.ActivationFunctionType
    OP = mybir.AluOpType
    k2t1 = sum(k2l1)
    k2t2 = sum(k2l2)

    nc = Bacc(num_devices=P)

    xp_in = nc.declare_dram_parameter("xp", [npad, D], bf16, isOutput=False)
    dis_in = nc.declare_dram_parameter("dis_pm", [128, b], f32, isOutput=False)
    w1_in = nc.declare_dram_parameter("W1", [D, D], f32, isOutput=False)
    w2_in = nc.declare_dram_parameter("W2", [D, D], f32, isOutput=False)
    b1t_in = nc.declare_dram_parameter("b1_tile", [D, D], f32, isOutput=False)
    b2t_in = nc.declare_dram_parameter("b2_tile", [D, D], f32, isOutput=False)
    iota_in = nc.declare_dram_parameter(
        "iota_rep", [128, 128 * k2tmax], bf16, isOutput=False
    )
    identb_in = nc.declare_dram_parameter("ident_bf", [128, 128], bf16, isOutput=False)
    idx1lo_in = nc.declare_dram_parameter(
        "idx1_lo", [128, g * gs * k2l1[0] * 8], i16, isOutput=False
    )
    idx1hi_in = nc.declare_dram_parameter(
        "idx1_hi", [128, g * gs * k2l1[1] * 8], i16, isOutput=False
    )
    idx2_in = [
        nc.declare_dram_parameter(
            f"idx2_{k}", [128, g * gs * k2l2[k] * 8], i16, isOutput=False
        )
        for k in range(NCHUNK)
    ]
    drel1_in = nc.declare_dram_parameter("drel1", [128, b * k2t1], bf16, isOutput=False)
    drel2_in = nc.declare_dram_parameter("drel2", [128, b * k2t2], bf16, isOutput=False)
    out = nc.declare_dram_parameter("out", [128, b, D], f32, isOutput=True)

    # AG chunk tensors: in = [128, bk, 128] partition-major; out stacks cores
    fp8 = mybir.dt.float8e4
    xdt = fp8 if FP8X else bf16
    x2own = []
    x2chunk = []
    x2bf = []
    for k, (b0, b1) in enumerate(cb):
        bk = b1 - b0
        x2own.append(nc.dram_tensor(f"x2own_{k}", [128, bk, D], xdt))
        x2chunk.append(
            nc.dram_tensor(f"x2chunk_{k}", [P, 128, bk, D], xdt, addr_space="Shared")
        )
        if FP8X:
            x2bf.append(nc.dram_tensor(f"x2bf_{k}", [P, 128, bk, D], bf16))
        else:
            x2bf.append(x2chunk[k])

    with TileContext(nc) as tc:
        with (
            tc.tile_pool(name="const", bufs=1) as const,
            tc.tile_pool(name="msgs", bufs=5) as msgs,
            tc.tile_pool(name="spool", bufs=3) as spool,
            tc.tile_pool(name="yout", bufs=3) as yout,
            tc.tile_pool(name="epi", bufs=8) as epi,
            tc.tile_pool(name="upc", bufs=2) as upc,
            tc.tile_pool(name="fin", bufs=2) as fin,
            tc.tile_pool(name="pa", bufs=4, space="PSUM") as pa,
            tc.tile_pool(name="pt", bufs=2, space="PSUM") as pt,
            tc.tile_pool(name="pz", bufs=2, space="PSUM") as pz,
        ):
            # ---- constants -------------------------------------------------
            def load_const(param, shape, dtype, tag):
                t = const.tile(shape, dtype, tag=tag)
                nc.sync.dma_start(t[:], param[:])
                return t

            # L1-critical consts first: the first gather waits on its idx
            # table; everything L2-only is loaded after L1 is emitted.
            idx1lo_sb = load_const(
                idx1lo_in, [128, g * gs * k2l1[0] * 8], i16, "idx1lo"
            )
            idx1hi_sb = load_const(
                idx1hi_in, [128, g * gs * k2l1[1] * 8], i16, "idx1hi"
            )
            drel1_sb = load_const(drel1_in, [128, b * k2t1], bf16, "drel1")
            iota_sb = load_const(iota_in, [128, 128 * k2tmax], bf16, "iota")
            dis_sb = load_const(dis_in, [128, b], f32, "dis")
            w1_sb = load_const(w1_in, [D, D], f32, "w1")
            b1t_sb = load_const(b1t_in, [D, D], f32, "b1t")
            identb_sb = load_const(identb_in, [128, 128], bf16, "identb")
            # L2-final consts: keep their DMA off the critical early window
            with tc.tile_wait_until(0.25):
                w2_sb = load_const(w2_in, [D, D], f32, "w2")
                b2t_sb = load_const(b2t_in, [D, D], f32, "b2t")

            w1b = const.tile([D, D], bf16, tag="w1b")
            nc.vector.tensor_copy(w1b[:], w1_sb[:])
            w2b = const.tile([D, D], bf16, tag="w2b")
            nc.vector.tensor_copy(w2b[:], w2_sb[:])

            # f32 partial for layer-2 aggregation (aggT layout [d, j] per block)
            partial = const.tile([128, b * 128], f32, tag="partial")

            # msg tile: sized for the largest gather unit actually issued
            mcols = max(4 * k2l1[0], 4 * k2l1[1], gs * max(k2l2))

            def gather(idx_sb, src_ap, k2h, gg, j0=0, nblk=gs):
                # gather chunks for blocks [gg*gs+j0, gg*gs+j0+nblk)
                L = nblk * k2h * 128
                l16g = gs * k2h * 8  # idx cols per group
                col0 = gg * l16g + j0 * k2h * 8
                msg = msgs.tile([128, mcols, D], bf16, tag="msg")
                nc.gpsimd.dma_gather(
                    msg[:, 0 : nblk * k2h, :],
                    src_ap,
                    idx_sb[:, col0 : col0 + L // 16],
                    L,
                    L,
                    D,
                    single_packet=False,
                )
                return msg

            def build_S(drel_sb, k2t, bb, koff=0, nch=None):
                # one-hot S slice for chunks [koff, koff+nch) of block bb
                if nch is None:
                    nch = k2t
                S = spool.tile([128, 128, k2tmax], bf16, tag="S")
                nc.vector.tensor_tensor(
                    S[:, :, 0:nch],
                    iota_sb[:, :].rearrange("p (j c) -> p j c", j=128)[:, :, 0:nch],
                    drel_sb[:, bb * k2t + koff : bb * k2t + koff + nch]
                    .rearrange("p (a c) -> p a c", a=1)
                    .broadcast_to([128, 128, nch]),
                    OP.is_equal,
                )
                return S

            # ---- layer 1 ---------------------------------------------------
            # per group: gather lo+hi, per block: S, 19 matmuls -> aggT PSUM,
            # epilogue -> x2stage [j, hid] bf16, write per chunk
            k2lo, k2hi = k2l1

            def l1_unit(gg, j0, nblk):
                mlo = gather(idx1lo_sb, xp_in[0:LO_SPLIT, :], k2lo, gg, j0, nblk)
                mhi = gather(idx1hi_sb, xp_in[LO_SPLIT:npad, :], k2hi, gg, j0, nblk)
                ystage = yout.tile([128, gs, D], fp8 if FP8X else bf16, tag="yst")
                for j in range(nblk):
                    bb = gg * gs + j0 + j
                    S = build_S(drel1_sb, k2t1, bb)
                    aggT = pa.tile([128, D], f32, tag="aggT")
                    for c in range(k2lo):
                        nc.tensor.matmul(
                            aggT[:],
                            mlo[:, j * k2lo + c, :],
                            S[:, :, c],
                            start=(c == 0),
                            stop=False,
                        )
                    for c in range(k2hi):
                        nc.tensor.matmul(
                            aggT[:],
                            mhi[:, j * k2hi + c, :],
                            S[:, :, k2lo + c],
                            start=False,
                            stop=(c == k2hi - 1),
                        )
                    # z = W1^T aggT  [hid, j]
                    aggb = epi.tile([128, D], bf16, tag="aggb")
                    nc.scalar.activation(aggb[:], aggT[:], AF.Copy)
                    z_p = pz.tile([128, D], f32, tag="z_p")
                    nc.tensor.matmul(z_p[:], w1b[:], aggb[:], start=True, stop=True)
                    zs = epi.tile([128, D], bf16, tag="zs")
                    nc.scalar.activation(zs[:], z_p[:], AF.Copy)
                    zT_p = pt.tile([128, D], bf16, tag="zT_p")
                    nc.tensor.transpose(zT_p[:], zs[:], identb_sb[:])
                    # tmp = dis_j * zT + b1_tile ; x2 = relu(dis_j * tmp)
                    tmp = epi.tile([128, D], f32, tag="tmp")
                    nc.vector.scalar_tensor_tensor(
                        tmp[:],
                        zT_p[:],
                        dis_sb[:, bb : bb + 1],
                        b1t_sb[:],
                        OP.mult,
                        OP.add,
                    )
                    nc.scalar.activation(
                        ystage[:, j, :],
                        tmp[:],
                        AF.Relu,
                        scale=dis_sb[:, bb : bb + 1],
                    )
                # write ystage blocks into their chunks (partition-major);
                # high priority so the write preempts queued gathers on the
                # DMA engines (it gates the serial AllGather chain)
                b0u = gg * gs + j0
                ju = 0
                with tc.high_priority():
                    while ju < nblk:
                        blk = b0u + ju
                        k = next(
                            i for i, (c0, c1) in enumerate(cb) if c0 <= blk < c1
                        )
                        c0, c1 = cb[k]
                        take = min(nblk - ju, c1 - blk)
                        nc.sync.dma_start(
                            x2own[k][:, blk - c0 : blk - c0 + take, :],
                            ystage[:, ju : ju + take, :],
                        )
                        ju += take

            # ---- layer 2 region pass --------------------------------------
            def load_own_group(gg):
                # own block rows (self-loop sources), cast to bf16 for PE
                xgrp = yout.tile([128, gs, D], fp8 if FP8X else bf16, tag="xgrp")
                b0g = gg * gs
                j0 = 0
                while j0 < gs:
                    blk = b0g + j0
                    k = next(i for i, (c0, c1) in enumerate(cb) if c0 <= blk < c1)
                    c0, c1 = cb[k]
                    take = min(gs - j0, c1 - blk)
                    nc.sync.dma_start(
                        xgrp[:, j0 : j0 + take, :],
                        x2own[k][:, blk - c0 : blk - c0 + take, :],
                    )
                    j0 += take
                if not FP8X:
                    return xgrp
                xgb = yout.tile([128, gs, D], bf16, tag="xgb")
                nc.scalar.activation(xgb[:, :, :], xgrp[:, :, :], AF.Copy)
                return xgb

            def upconvert(k):
                # fp8 AG output -> bf16 gather source, per sender core slice
                bk = cb[k][1] - cb[k][0]
                for cc in range(P):
                    t8 = upc.tile([128, 20, D], fp8, tag="u8")
                    nc.sync.dma_start(t8[:, 0:bk, :], x2chunk[k][cc, :, :, :])
                    tb = upc.tile([128, 20, D], bf16, tag="ub")
                    nc.scalar.activation(tb[:, 0:bk, :], t8[:, 0:bk, :], AF.Copy)
                    nc.sync.dma_start(x2bf[k][cc, :, :, :], tb[:, 0:bk, :])

            def l2_region(k, gg, first):
                k2r = k2l2[k]
                koff = sum(k2l2[:k])
                src = x2bf[k][:, :, :, :].rearrange("c p b d -> (c p b) d")
                m = gather(idx2_sb[k], src, k2r, gg)
                xgrp = load_own_group(gg) if first else None
                last = k == NCHUNK - 1
                for j in range(gs):
                    bb = gg * gs + j
                    if last:
                        S = s3all[:, :, bb * k2r : (bb + 1) * k2r]
                    else:
                        S = build_S(drel2_sb, k2t2, bb, koff=koff, nch=k2r)
                    aggT = pa.tile([128, D], f32, tag="aggT")
                    if first:
                        # self-loop contribution: aggT[d, j'] += xgrp[j', d]
                        # (stat=xgrp, mov=identity == transpose, f32 accum)
                        nc.tensor.matmul(
                            aggT[:],
                            xgrp[:, j, :],
                            identb_sb[:],
                            start=True,
                            stop=False,
                        )
                    for c in range(k2r):
                        nc.tensor.matmul(
                            aggT[:],
                            m[:, j * k2r + c, :],
                            S[:, :, c],
                            start=(not first) and (c == 0),
                            stop=(c == k2r - 1),
                        )
                    if first:
                        nc.scalar.activation(
                            partial[:, bb * 128 : (bb + 1) * 128], aggT[:], AF.Copy
                        )
                    elif k < NCHUNK - 1:
                        nc.vector.scalar_tensor_tensor(
                            partial[:, bb * 128 : (bb + 1) * 128],
                            aggT[:],
                            1.0,
                            partial[:, bb * 128 : (bb + 1) * 128],
                            OP.mult,
                            OP.add,
                        )
                    else:
                        # last region: merge straight into the bf16 matmul
                        # input (saves a partial round-trip in the tail)
                        aggb = fin.tile([128, D], bf16, tag=f"aggf{j}")
                        nc.vector.scalar_tensor_tensor(
                            aggb[:],
                            aggT[:],
                            1.0,
                            partial[:, bb * 128 : (bb + 1) * 128],
                            OP.mult,
                            OP.add,
                        )
                        aggfinal[j] = aggb

            def l2_final(gg):
                ostage = yout.tile([128, gs, D], f32, tag="ost")
                for j in range(gs):
                    bb = gg * gs + j
                    aggb = aggfinal[j]
                    z_p = pz.tile([128, D], f32, tag="z_p")
                    nc.tensor.matmul(z_p[:], w2b[:], aggb[:], start=True, stop=True)
                    zs = epi.tile([128, D], bf16, tag="zs")
                    nc.scalar.activation(zs[:], z_p[:], AF.Copy)
                    zT_p = pt.tile([128, D], bf16, tag="zT_p")
                    nc.tensor.transpose(zT_p[:], zs[:], identb_sb[:])
                    nc.vector.scalar_tensor_tensor(
                        ostage[:, j, :],
                        zT_p[:],
                        dis_sb[:, bb : bb + 1],
                        b2t_sb[:],
                        OP.mult,
                        OP.add,
                    )
                nc.sync.dma_start(
                    out[:, gg * gs : (gg + 1) * gs, :], ostage[:, :, :]
                )

            # ---- schedule: L1 units, AGs as chunks complete, L2 regions --
            done_chunk = [False] * NCHUNK
            blocks_done = 0

            def group_units(gg):
                # unit boundaries: chunk edges (so AGs fire promptly) plus
                # cuts keeping units <= 4 blocks
                lo, hi = gg * gs, (gg + 1) * gs
                cuts = {lo, hi}
                for (c0, c1) in cb:
                    if lo < c1 < hi:
                        cuts.add(c1)
                pts = sorted(cuts)
                units = []
                for a, bnd in zip(pts, pts[1:]):
                    seg = bnd - a
                    while seg > 4:
                        half = (seg + 1) // 2 if seg <= 8 else 4
                        units.append(half)
                        seg -= half
                    units.append(seg)
                return units

            for gg in range(g):
                j0 = 0
                for nblk in group_units(gg):
                    l1_unit(gg, j0, nblk)
                    j0 += nblk
                    blocks_done += nblk
                    for k, (c0, c1) in enumerate(cb):
                        if not done_chunk[k] and blocks_done >= c1:
                            with tc.high_priority():
                                nc.gpsimd.collective_compute(
                                    "AllGather",
                                    mybir.AluOpType.bypass,
                                    replica_groups=[list(range(P))],
                                    ins=[x2own[k][:, :, :]],
                                    outs=[x2chunk[k][:, :, :, :]],
                                )
                            if FP8X:
                                with tc.tile_wait_until(L2_DELAY_MS - 0.01):
                                    upconvert(k)
                            done_chunk[k] = True
            # L2-only consts, staggered: each needed only when its AG lands
            idx2_sb = []
            for k in range(NCHUNK):
                with tc.tile_wait_until(0.035 + 0.08 * k):
                    idx2_sb.append(
                        load_const(
                            idx2_in[k], [128, g * gs * k2l2[k] * 8], i16, f"idx2_{k}"
                        )
                    )
            with tc.tile_wait_until(0.035):
                drel2_sb = load_const(drel2_in, [128, b * k2t2], bf16, "drel2")

            # prebuild ALL last-region S matrices in one early is_equal (the
            # tail otherwise pays 49 small DVE builds after the final AG)
            k2last = k2l2[NCHUNK - 1]
            kofflast = sum(k2l2[: NCHUNK - 1])
            s3all = const.tile([128, 128, b * k2last], bf16, tag="s3all")
            nc.vector.tensor_tensor(
                s3all[:, :, :].rearrange("p j (b c) -> p j b c", b=b),
                iota_sb[:, 0 : 128 * k2tmax]
                .rearrange("p (j x) -> p j x", j=128)[:, :, 0:k2last]
                .unsqueeze(2)
                .broadcast_to([128, 128, b, k2last]),
                drel2_sb[:, :]
                .rearrange("p (b x) -> p b x", b=b)[:, :, kofflast : kofflast + k2last]
                .unsqueeze(1)
                .broadcast_to([128, 128, b, k2last]),
                OP.is_equal,
            )
            aggfinal = [None] * gs
            for k in range(NCHUNK):
                # keep L2 region DMA out of layer 1's window: layer-1 gathers
                # pace the serial AllGather chain, so they get the DMA first
                with tc.tile_wait_until(L2_DELAY_MS, enable=FP8X):
                    for gg in range(g):
                        l2_region(k, gg, first=(k == 0))
                        if k == NCHUNK - 1:
                            l2_final(gg)

    nc.finalize()
    return nc


def make_in_maps(pl, x, w1, b1, w2, b2):
    n = x.shape[0]
    (rpc, npad, b, gs, g, cb, k2tmax, k2l1, k2l2) = pl["sizes"]
    dis = pl["dis"]
    xp = np.zeros((npad, D), dtype=BF16)
    xp[:n] = (x.astype(np.float32) * dis[:, None]).astype(BF16)
    shared = {
        "xp": xp,
        "W1": np.ascontiguousarray(w1.astype(np.float32)),
        "W2": np.ascontiguousarray(w2.astype(np.float32)),
        "b1_tile": np.ascontiguousarray(
            np.tile(b1.astype(np.float32).reshape(1, D), (D, 1))
        ),
        "b2_tile": np.ascontiguousarray(
            np.tile(b2.astype(np.float32).reshape(1, D), (D, 1))
        ),
        "iota_rep": pl["iota_rep"],
        "ident_bf": pl["ident_bf"],
    }
    in_maps = []
    for c in range(P):
        m = dict(shared)
        pc = pl["per_core"][c]
        m["dis_pm"] = pc["dis_pm"]
        m["idx1_lo"] = pc["idx1_lo"]
        m["idx1_hi"] = pc["idx1_hi"]
        m["drel1"] = pc["drel1"]
        m["drel2"] = pc["drel2"]
        for k in range(NCHUNK):
            m[f"idx2_{k}"] = pc[f"idx2_{k}"]
        in_maps.append(m)
    return in_maps


_CACHE = {}


def kernel(x, edge_index, W1, b1, W2, b2):
    from concourse.bass_utils import run_bass_kernel_spmd

    x = np.asarray(x)
    edge_index = np.asarray(edge_index)
    n = x.shape[0]
    pl = plan(edge_index, n)
    key = pl["sizes"]
    if key not in _CACHE:
        _CACHE[key] = build_program(pl)
    nc = _CACHE[key]
    in_maps = make_in_maps(
        pl, x, np.asarray(W1), np.asarray(b1), np.asarray(W2), np.asarray(b2)
    )
    last_err = None
    for backoff in (15.0, 45.0, 0.0):
        try:
            r = run_bass_kernel_spmd(nc, in_maps, list(range(P)))
            break
        except Exception as ex:  # transient NRT/axon failures wedge briefly
            last_err = ex
            if backoff:
                import time

                time.sleep(backoff)
    else:
        raise last_err

    perm_core, perm_blk, perm_slot = pl["perm"]
    outs = np.stack([r.results[c]["out"] for c in range(P)], axis=0)
    # outs[c][p, b, d] -> node rows
    res = outs[perm_core, perm_slot, perm_blk]
    return np.ascontiguousarray(res).astype(np.float32)
